# revision 24
# baseline (speedup 1.0000x reference)
"""CFG dual self-attention kernel for 8 Trainium2 NeuronCores.

Strategy (tensor parallel on heads):
  - h = concat(hidden_cond, hidden_uncond) -> [4096 tokens, 5120]; host
    pre-transposes to hT [5120, 4096] so the QKV matmul contraction dim (5120)
    lands on SBUF partitions.
  - Each core owns 5 heads (640 of the 5120 q/k/v channels).  It computes
    qT/kT [640, 4096] (transposed layout: head-dim on partitions) and
    v [4096, 640] (natural layout) from hT with fp32r matmuls.
  - RMSNorm over the full 5120 dims needs a cross-core sum of squares:
    partial ssq per token is computed with ones-matmuls on the PE and
    allreduced across the 8 cores (32 KB collective, hidden under the V
    projection).
  - Attention per (batch, head) in scores-transposed layout
    scoresT[st, sq] = (rope(k) slice)^T @ rope(q): softmax denominators via
    ones-matmul column sums (interleaved PSUM accumulation groups), exp on
    the scalar engine, A@V accumulated with v-chunks stationary, and the
    1/colsum normalization folded into the PSUM->SBUF eviction of attn_outT.
  - Output projection: partial_out = attn_outT^T @ Wout[rows of this core]
    -> [4096, 5120] per-core partial; host sums the 8 partials (+ bout).
"""

import numpy as np

import concourse.bass as bass  # noqa: F401  (bass types via bacc)
import concourse.mybir as mybir
import concourse.tile as tile
from concourse import bacc
from concourse.bass_utils import run_bass_kernel_spmd

F32 = mybir.dt.float32
F32R = mybir.dt.float32r

NCORES = 8
EPS = 1e-6


def _vblocks(cw):
    # split the v output width into matmul N-blocks (>=256 keeps fp32r fast)
    if cw % 320 == 0:
        return [320] * (cw // 320)
    return [cw]


def build_program(S, DIM, H, collective=True, repeat=1):
    """Emit the per-core bass program (identical on all cores; per-core data
    differences come entirely from the input tensors)."""
    HD = 128
    assert DIM == H * HD
    HPC = H // NCORES          # heads per core
    CW = HPC * HD              # per-core channel width for q/k/v
    CT = HPC                   # 128-col tiles per group
    NT = 2 * S                 # tokens across both batches
    DC = DIM // 128            # contraction chunks
    TBS = 256                  # token block in phase 1
    NTB = NT // TBS
    SQB = min(512, S)          # sq block in attention
    NSQ = S // SQB
    NST = S // 128             # st (key) chunks per batch
    VNB = _vblocks(CW)
    ONB = DIM // 512           # out-proj N blocks
    # contraction sub-blocks for merged DMA (dep granularity)
    CSUB = 10 if DC % 10 == 0 else DC
    NCS = DC // CSUB

    nc = bacc.Bacc("TRN2", target_bir_lowering=False, debug=False,
                   num_devices=NCORES)

    hT = nc.dram_tensor("hT", [DIM, NT], F32, kind="ExternalInput")
    wq = nc.dram_tensor("wq", [DIM, CW], F32, kind="ExternalInput")
    wk = nc.dram_tensor("wk", [DIM, CW], F32, kind="ExternalInput")
    wv = nc.dram_tensor("wv", [DIM, CW], F32, kind="ExternalInput")
    bq = nc.dram_tensor("bq", [128, CT], F32, kind="ExternalInput")
    bk = nc.dram_tensor("bk", [128, CT], F32, kind="ExternalInput")
    wqn = nc.dram_tensor("wqn", [128, CT], F32, kind="ExternalInput")
    wkn = nc.dram_tensor("wkn", [128, CT], F32, kind="ExternalInput")
    cosT = nc.dram_tensor("cosT", [128, S], F32, kind="ExternalInput")
    sinrT = nc.dram_tensor("sinrT", [128, S], F32, kind="ExternalInput")
    wout = nc.dram_tensor("wout", [CW, DIM], F32, kind="ExternalInput")
    outp = nc.dram_tensor("outp", [NT, DIM], F32, kind="ExternalOutput")

    hT3 = hT.rearrange("(c p) t -> p c t", p=128).bitcast(F32R)

    with tile.TileContext(nc) as tc:
        with (
            tc.tile_pool(name="dram", bufs=1, space="DRAM") as dram,
            tc.tile_pool(name="persist", bufs=1) as persist,
        ):
            for _rep in range(repeat):
              qsc = dram.tile([CW, NT], F32, tag="qsc")
              ksc = dram.tile([CW, NT], F32, tag="ksc")
              vsc = dram.tile([NT, CW], F32, tag="vsc")
              aosc = dram.tile([CW, NT], F32, tag="aosc")
              cc_in = dram.tile([2, NT], F32, tag="cc_in")
              cc_out = dram.tile([2, NT], F32, tag="cc_out")

              # constants
              ones_f = persist.tile([128, 1], F32, tag="ones_f")
              nc.vector.memset(ones_f[:], 1.0)
              ones = persist.tile([128, 1], F32R, tag="ones")
              nc.vector.tensor_copy(ones[:], ones_f[:])

              bq_t = persist.tile([128, CT], F32, tag="bq")
              nc.sync.dma_start(bq_t[:], bq[:])
              bk_t = persist.tile([128, CT], F32, tag="bk")
              nc.sync.dma_start(bk_t[:], bk[:])
              wqn_t = persist.tile([128, CT], F32, tag="wqn")
              nc.sync.dma_start(wqn_t[:], wqn[:])
              wkn_t = persist.tile([128, CT], F32, tag="wkn")
              nc.sync.dma_start(wkn_t[:], wkn[:])

              # pre-reserved pool for the b=0 norm-factor tiles: opened
              # before the phase-1 pools and closed after phase 3, so their
              # DMAs/broadcasts never WAR-depend on recycled phase-1
              # addresses and can run during the v group
              from contextlib import ExitStack as _ES
              _rb_ctx = _ES()
              rb0p = _rb_ctx.enter_context(tc.tile_pool(name="rb0p", bufs=1))

              # ---------------- phase 1: qkv projections + ssq partials -------
              with (
                  tc.tile_pool(name="wp", bufs=1) as wp,
                  tc.tile_pool(name="hp", bufs=2) as hp,
                  tc.tile_pool(name="ev", bufs=1) as evp,
              ):
                  def load_hall(tb):
                      hall = hp.tile([128, DC, TBS], F32R, tag="hall")
                      for cs in range(NCS):
                          nc.sync.dma_start(
                              hall[:, cs * CSUB:(cs + 1) * CSUB, :],
                              hT3[:, cs * CSUB:(cs + 1) * CSUB,
                                  tb * TBS:(tb + 1) * TBS])
                      return hall

                  def load_wall(wdram, interleave_tb0=None, order="ct"):
                      # sub-loads ordered to match the consumer loop: ct-major
                      # for q/k (col-tile outer), chunk-major for v (chunk
                      # outer) — the first matmul group only waits for the
                      # first slice, not the whole W block
                      wall = wp.tile([128, DC, CW], F32R, tag="wall")
                      w3 = wdram.rearrange("(c p) n -> p c n", p=128) \
                          .bitcast(F32R)
                      hall0 = None
                      if interleave_tb0 is not None:
                          hall0 = hp.tile([128, DC, TBS], F32R, tag="hall")
                      if order == "ct":
                          wslices = [(slice(None), slice(ct * 128,
                                                         (ct + 1) * 128))
                                     for ct in range(CT)]
                      else:
                          wslices = [(slice(cs * CSUB, (cs + 1) * CSUB),
                                      slice(None)) for cs in range(NCS)]
                      for si, (csl, nsl) in enumerate(wslices):
                          nc.sync.dma_start(wall[:, csl, nsl],
                                            w3[:, csl, nsl])
                          if hall0 is not None and si < NCS:
                              nc.sync.dma_start(
                                  hall0[:, si * CSUB:(si + 1) * CSUB, :],
                                  hT3[:, si * CSUB:(si + 1) * CSUB, 0:TBS])
                      if hall0 is not None:
                          for cs in range(len(wslices), NCS):
                              nc.sync.dma_start(
                                  hall0[:, cs * CSUB:(cs + 1) * CSUB, :],
                                  hT3[:, cs * CSUB:(cs + 1) * CSUB, 0:TBS])
                      return wall, hall0

                  for gi, (wdram, bias_t, spill) in enumerate(
                      [(wq, bq_t, qsc), (wk, bk_t, ksc)]
                  ):
                      with (
                          tc.tile_pool(name=f"ps{gi}", bufs=4, space="PSUM") as psp,
                          tc.tile_pool(name=f"sq{gi}", bufs=2, space="PSUM") as sqp,
                          tc.tile_pool(name=f"aux{gi}", bufs=1) as st1,
                      ):
                          wall, hall0 = load_wall(wdram, interleave_tb0=True)
                          deferred = []
                          for tb in range(NTB):
                              hall = hall0 if tb == 0 else load_hall(tb)
                              hall0 = None
                              ssq_ps = sqp.tile([1, TBS], F32, tag="ssq")
                              evq = evp.tile([128, CT, TBS], F32R, tag="evq")
                              sqt = st1.tile([128, CT, TBS], F32R, tag="sqt")
                              for ct in range(CT):
                                  pq = psp.tile([128, TBS], F32, tag="acc")
                                  for ch in range(DC):
                                      nc.tensor.matmul(
                                          pq[:],
                                          wall[:, ch, ct * 128:(ct + 1) * 128],
                                          hall[:, ch, :],
                                          start=(ch == 0), stop=(ch == DC - 1))
                                  # the ssq matmul of the PREVIOUS ct group goes
                                  # here so the PE never waits on the DVE square
                                  if deferred:
                                      deferred.pop(0)()
                                  nc.vector.tensor_scalar_add(
                                      evq[:, ct, :], pq[:], bias_t[:, ct:ct + 1])
                                  nc.sync.dma_start(
                                      spill.rearrange("(c p) t -> p c t", p=128)
                                      .bitcast(F32R)
                                      [:, ct, tb * TBS:(tb + 1) * TBS],
                                      evq[:, ct, :])
                                  nc.vector.tensor_mul(
                                      sqt[:, ct, :], evq[:, ct, :].bitcast(F32),
                                      evq[:, ct, :].bitcast(F32))

                                  def emit_ssq(ssq_ps=ssq_ps, sqt=sqt, ct=ct,
                                               evq=evq, tb=tb, spill=spill,
                                               gi=gi):
                                      nc.tensor.matmul(
                                          ssq_ps[:], ones[:], sqt[:, ct, :],
                                          start=(ct == 0), stop=(ct == CT - 1))
                                      if ct == CT - 1:
                                          stg = st1.tile([1, TBS], F32,
                                                         tag="stg", name="stg")
                                          nc.vector.tensor_copy(stg[:],
                                                                ssq_ps[:])
                                          nc.gpsimd.dma_start(
                                              cc_in[gi:gi + 1,
                                                    tb * TBS:(tb + 1) * TBS],
                                              stg[:])
                                  deferred.append(emit_ssq)
                          while deferred:
                              deferred.pop(0)()

                  # allreduce the ssq partials (overlaps with the v group below)
                  if collective:
                      nc.gpsimd.collective_compute(
                          "AllReduce", mybir.AluOpType.add,
                          replica_groups=[list(range(NCORES))],
                          ins=[cc_in[:].opt()], outs=[cc_out[:].opt()])
                  else:
                      # single-core timing-sim variant: stand-in for the
                      # allreduce so TimelineSim (no collectives) can run
                      nc.sync.dma_start(cc_out[:], cc_in[:])

                  # v projection (natural layout, hT token-tiles stationary)
                  with tc.tile_pool(name="psv", bufs=2, space="PSUM") as psv:
                      wall, hall0 = load_wall(wv, interleave_tb0=True, order="cs")
                      for tb in range(NTB):
                          hall = hall0 if tb == 0 else load_hall(tb)
                          hall0 = None
                          nsub = TBS // 128
                          pv = [[psv.tile([128, nb], F32, tag=f"pv{ts}_{i}",
                                          name=f"pv{ts}_{i}")
                                 for i, nb in enumerate(VNB)]
                                for ts in range(nsub)]
                          for ch in range(DC):
                              for ts in range(nsub):
                                  off = 0
                                  for i, nb in enumerate(VNB):
                                      nc.tensor.matmul(
                                          pv[ts][i][:],
                                          hall[:, ch, ts * 128:(ts + 1) * 128],
                                          wall[:, ch, off:off + nb],
                                          start=(ch == 0), stop=(ch == DC - 1))
                                      off += nb
                          evv = evp.tile([128, nsub, CW], F32R, tag="evq",
                                         name="evv")
                          for ts in range(nsub):
                              off = 0
                              for i, nb in enumerate(VNB):
                                  # v bias is folded into the host-side output
                                  # bias (softmax rows sum to 1), so this is a
                                  # pure PSUM->SBUF copy on the idle scalar
                                  # engine, keeping DVE free for the rinv/rope
                                  # chains that overlap the v group
                                  nc.scalar.copy(
                                      evv[:, ts, off:off + nb], pv[ts][i][:])
                                  off += nb
                              nc.sync.dma_start(
                                  vsc.rearrange("(b p) n -> p b n", p=128)
                                  .bitcast(F32R)[:, tb * nsub + ts, :],
                                  evv[:, ts, :])

              # ---------------- phase 3: attention per (batch, head) ----------
              # prefix of Wout loaded into its own (non-overlapping) pool while
              # attention still runs, so phase 4 starts without a DMA stall
              WOSL = 1024 if DIM >= 2048 else 512
              w3o = wout.rearrange("(c p) n -> p c n", p=128).bitcast(F32R)
              with tc.tile_pool(name="wo0p", bufs=1) as wo0p:
                with (
                  tc.tile_pool(name="p3", bufs=2) as p3,
                  tc.tile_pool(name="p3e", bufs=3) as p3e,
                  tc.tile_pool(name="cs3", bufs=1) as cs3,
                  tc.tile_pool(name="ps_sc", bufs=3, space="PSUM") as ps_sc,
                  tc.tile_pool(name="ps_cs", bufs=2, space="PSUM") as ps_cs,
                  tc.tile_pool(name="ps_av", bufs=2, space="PSUM") as ps_av,
                ):
                  cosT_t = cs3.tile([128, S], F32, tag="cosT")
                  nc.sync.dma_start(cosT_t[:], cosT[:])
                  sinrT_t = cs3.tile([128, S], F32, tag="sinrT")
                  nc.sync.dma_start(sinrT_t[:], sinrT[:])
                  wot0 = wo0p.tile([128, HPC, WOSL], F32R, tag="wot0")

                  # rinv = 1/sqrt(mean ssq + eps): broadcast the allreduced
                  # ssq rows first, then do the math 128-lane-parallel.
                  # Only the b=0 factors are on the critical path; b=1 is
                  # deferred into the first pair's attention.
                  rb = {}

                  def emit_rb(b):
                      for gi in range(2):
                          # fold the HD**-0.5 attention scale into the q side:
                          # s/sqrt(ssq/DIM+eps) == 1/sqrt(ssq*HD/DIM+HD*eps)
                          sc1 = (HD / DIM) if gi == 0 else (1.0 / DIM)
                          sc2 = (HD * EPS) if gi == 0 else EPS
                          row = p3.tile([1, S], F32, tag="rrow", name="rrow")
                          nc.sync.dma_start(
                              row[:], cc_out[gi:gi + 1, b * S:(b + 1) * S])
                          t = cs3.tile([128, S], F32, tag=f"rb{gi}{b}",
                                       name=f"rb{gi}{b}")
                          nc.gpsimd.partition_broadcast(t[:], row[:])
                          nc.vector.tensor_scalar(t[:], t[:], sc1, sc2,
                                                  mybir.AluOpType.mult,
                                                  mybir.AluOpType.add)
                          nc.scalar.activation(
                              t[:], t[:], mybir.ActivationFunctionType.Sqrt)
                          nc.vector.reciprocal(t[:], t[:])
                          rb[(gi, b)] = t

                  emit_rb(0)

                  bhs = [(b, hh) for b in range(2) for hh in range(HPC)]

                  def prep(i, nchunk=1):
                      """Load + norm + rope q/k and load v for pair i.  Emitted
                      one pair ahead so the DVE work overlaps the previous
                      pair's attention matmuls.  nchunk>1 slices the chain
                      column-wise so the first attention matmul only waits
                      for the first slice (used for the first pair, whose
                      chain cannot overlap anything earlier)."""
                      b, hh = bhs[i]
                      CS2 = S // nchunk
                      qkr = []
                      for gi, (spill, wn) in enumerate(
                              [(qsc, wqn_t), (ksc, wkn_t)]):
                          xt = p3.tile([128, S], F32, tag="xt", name="xt")
                          tmc = p3.tile([128, S], F32, tag="tmc", name="tmc")
                          tms = p3.tile([128, S], F32, tag="tms", name="tms")
                          xr = p3.tile([128, S], F32R, tag="xr", name="xr")
                          for cc in range(nchunk):
                              sl = slice(cc * CS2, (cc + 1) * CS2)
                              nc.sync.dma_start(
                                  xt[:, sl],
                                  spill[hh * 128:(hh + 1) * 128,
                                        b * S + cc * CS2:
                                        b * S + (cc + 1) * CS2])
                              nc.vector.tensor_mul(xt[:, sl], xt[:, sl],
                                                   rb[(gi, b)][:, sl])
                              nc.vector.tensor_scalar_mul(
                                  xt[:, sl], xt[:, sl], wn[:, hh:hh + 1])
                              nc.vector.tensor_mul(tmc[:, sl], xt[:, sl],
                                                   cosT_t[:, sl])
                              nc.vector.tensor_mul(
                                  tms[0:64, sl], xt[64:128, sl],
                                  sinrT_t[64:128, sl])
                              nc.vector.tensor_mul(
                                  tms[64:128, sl], xt[0:64, sl],
                                  sinrT_t[0:64, sl])
                              nc.vector.tensor_add(xr[:, sl], tmc[:, sl],
                                                   tms[:, sl])
                          qkr.append(xr)
                      vt = p3.tile([128, NST, 128], F32R, tag="vt", name="vt")
                      nc.gpsimd.dma_start(
                          vt[:], vsc[b * S:(b + 1) * S,
                                     hh * 128:(hh + 1) * 128]
                          .bitcast(F32R)
                          .rearrange("(c p) d -> p c d", p=128))
                      return qkr[0], qkr[1], vt

                  preps = {0: prep(0, nchunk=4)}
                  for i in range(len(bhs)):
                      b, hh = bhs[i]
                      if i + 1 < len(bhs) and i > 0:
                          preps[i + 1] = prep(i + 1)
                      qr, kr, vt = preps.pop(i)
                      if True:
                          ao = p3.tile([128, S], F32R, tag="ao")
                          for sqb in range(NSQ):
                              cs = ps_cs.tile([1, SQB], F32, tag="cs")
                              av = ps_av.tile([128, SQB], F32, tag="av")
                              # one-step lookahead: scores MM for st+1 issues
                              # before the cs/av MMs of st, hiding exp latency
                              ets = {}
                              for st in range(NST + 1):
                                  if st < NST:
                                      sc = ps_sc.tile([128, SQB], F32, tag="sc")
                                      nc.tensor.matmul(
                                          sc[:], kr[:, st * 128:(st + 1) * 128],
                                          qr[:, sqb * SQB:(sqb + 1) * SQB],
                                          start=True, stop=True)
                                      et = p3e.tile([128, SQB], F32R, tag="et")
                                      nc.scalar.activation(
                                          et[:], sc[:],
                                          mybir.ActivationFunctionType.Exp)
                                      ets[st] = et
                                  if st >= 1:
                                      pst = st - 1
                                      et = ets.pop(pst)
                                      nc.tensor.matmul(
                                          cs[:], ones[:], et[:],
                                          start=(pst == 0),
                                          stop=(pst == NST - 1))
                                      nc.tensor.matmul(
                                          av[:], vt[:, pst, :], et[:],
                                          start=(pst == 0),
                                          stop=(pst == NST - 1))
                              rc = p3.tile([1, SQB], F32, tag="rc")
                              nc.vector.reciprocal(rc[:], cs[:])
                              rb2 = p3.tile([128, SQB], F32, tag="rb2")
                              nc.gpsimd.partition_broadcast(rb2[:], rc[:])
                              nc.vector.tensor_mul(
                                  ao[:, sqb * SQB:(sqb + 1) * SQB],
                                  av[:], rb2[:])
                              if i == 0 and sqb == 0:
                                  # off the critical path: the b=1 norm
                                  # factors and the second pair's prep
                                  emit_rb(1)
                                  preps[1] = prep(1)
                                  # prefetch of the Wout prefix for phase 4
                                  for nb in range(WOSL // 512):
                                      nc.sync.dma_start(
                                          wot0[:, :, nb * 512:(nb + 1) * 512],
                                          w3o[:, :, nb * 512:(nb + 1) * 512])
                          nc.gpsimd.dma_start(
                              aosc[hh * 128:(hh + 1) * 128,
                                   b * S:(b + 1) * S].bitcast(F32R), ao[:])

                # -------------- phase 4: partial output projection ----------
                NWO0 = WOSL // 512
                with (
                  tc.tile_pool(name="wo", bufs=1) as wo,
                  tc.tile_pool(name="p4", bufs=4) as p4,
                  tc.tile_pool(name="oe", bufs=2) as oep,
                  tc.tile_pool(name="ps4", bufs=4, space="PSUM") as ps4,
                ):
                  def load_aot(tt):
                      aot = p4.tile([128, HPC, 128], F32R, tag="aot",
                                    name="aot")
                      nc.sync.dma_start(
                          aot[:], aosc[:, tt * 128:(tt + 1) * 128].bitcast(F32R)
                          .rearrange("(c p) n -> p c n", p=128))
                      return aot

                  aot_next = load_aot(0)   # first attn-out tile before W bulk
                  wot = None
                  if DIM > WOSL:
                      wot = wo.tile([128, HPC, DIM - WOSL], F32R, tag="wot")
                      # nb-major sub-loads: each (tt, nb) group only waits for
                      # one 512-column slice of Wout
                      for nb in range(NWO0, ONB):
                          nc.sync.dma_start(
                              wot[:, :, (nb - NWO0) * 512:
                                  (nb - NWO0 + 1) * 512],
                              w3o[:, :, nb * 512:(nb + 1) * 512])
                  for tt in range(NT // 128):
                      aot = aot_next
                      if tt + 1 < NT // 128:
                          aot_next = load_aot(tt + 1)
                      orow = oep.tile([128, DIM], F32, tag="orow")
                      for nb in range(ONB):
                          if nb < NWO0:
                              wsl = wot0[:, :, nb * 512:(nb + 1) * 512]
                          else:
                              wsl = wot[:, :, (nb - NWO0) * 512:
                                        (nb - NWO0 + 1) * 512]
                          po = ps4.tile([128, 512], F32, tag="po")
                          for ch in range(HPC):
                              nc.tensor.matmul(
                                  po[:], aot[:, ch, :], wsl[:, ch, :],
                                  start=(ch == 0), stop=(ch == HPC - 1))
                          nc.vector.tensor_copy(
                              orow[:, nb * 512:(nb + 1) * 512], po[:])
                      nc.sync.dma_start(
                          outp[tt * 128:(tt + 1) * 128, :], orow[:])
              _rb_ctx.close()
    nc.finalize()
    return nc


_PROGRAM_CACHE = {}


def _get_program(S, DIM, H):
    key = (S, DIM, H)
    if key not in _PROGRAM_CACHE:
        _PROGRAM_CACHE[key] = build_program(S, DIM, H)
    return _PROGRAM_CACHE[key]


def make_in_maps(S, DIM, H, hidden_cond, hidden_uncond, cos_freqs, sin_freqs,
                 Wqkv, bqkv, wq_norm, wk_norm, Wout, bout):
    HD = 128
    HPC = H // NCORES
    CW = HPC * HD
    h = np.concatenate([np.asarray(hidden_cond), np.asarray(hidden_uncond)],
                       axis=0).reshape(2 * S, DIM)
    hT = np.ascontiguousarray(h.T)
    cosT = np.ascontiguousarray(np.asarray(cos_freqs).T)
    sinT = np.asarray(sin_freqs).T  # [128, S]
    HF = HD // 2
    sinrT = np.concatenate([sinT[HF:], -sinT[:HF]], axis=0)
    sinrT = np.ascontiguousarray(sinrT)
    Wqkv = np.asarray(Wqkv)
    bqkv = np.asarray(bqkv)
    wq_norm = np.asarray(wq_norm)
    wk_norm = np.asarray(wk_norm)
    Wout = np.asarray(Wout)

    in_maps = []
    for c in range(NCORES):
        sl = slice(c * CW, (c + 1) * CW)
        bq_c = bqkv[0 * DIM:1 * DIM][sl].reshape(HPC, HD).T
        bk_c = bqkv[1 * DIM:2 * DIM][sl].reshape(HPC, HD).T
        in_maps.append({
            "hT": hT,
            "wq": np.ascontiguousarray(Wqkv[:, 0 * DIM:1 * DIM][:, sl]),
            "wk": np.ascontiguousarray(Wqkv[:, 1 * DIM:2 * DIM][:, sl]),
            "wv": np.ascontiguousarray(Wqkv[:, 2 * DIM:3 * DIM][:, sl]),
            "bq": np.ascontiguousarray(bq_c),
            "bk": np.ascontiguousarray(bk_c),
            "wqn": np.ascontiguousarray(wq_norm[sl].reshape(HPC, HD).T),
            "wkn": np.ascontiguousarray(wk_norm[sl].reshape(HPC, HD).T),
            "cosT": cosT,
            "sinrT": sinrT,
            "wout": np.ascontiguousarray(Wout[sl, :]),
        })
    return in_maps


def run(S, DIM, H, inputs):
    nc = _get_program(S, DIM, H)
    in_maps = make_in_maps(S, DIM, H, **inputs)
    res = run_bass_kernel_spmd(nc, in_maps, list(range(NCORES)))
    partial = np.zeros((2 * S, DIM), np.float64)
    for r in res.results:
        partial += r["outp"].astype(np.float64)
    # the v-bias contribution: softmax rows sum to 1, so attn(v + 1*bv) =
    # attn(v) + 1*bv, and bv flows through Wout as a constant per-channel term
    bv_full = np.asarray(inputs["bqkv"])[2 * DIM:3 * DIM].astype(np.float64)
    const_bias = bv_full @ np.asarray(inputs["Wout"]).astype(np.float64) \
        + np.asarray(inputs["bout"])
    out = (partial + const_bias[None, :]).astype(np.float32)
    out = out.reshape(2, 1, S, DIM)
    return out[0], out[1]


def kernel(hidden_cond, hidden_uncond, cos_freqs, sin_freqs,
           Wqkv, bqkv, wq_norm, wk_norm, Wout, bout):
    B, S, DIM = np.asarray(hidden_cond).shape
    assert B == 1
    H = DIM // 128
    return run(S, DIM, H, dict(
        hidden_cond=hidden_cond, hidden_uncond=hidden_uncond,
        cos_freqs=cos_freqs, sin_freqs=sin_freqs, Wqkv=Wqkv, bqkv=bqkv,
        wq_norm=wq_norm, wk_norm=wk_norm, Wout=Wout, bout=bout))



# revision 27
# speedup vs baseline: 1.1597x; 1.1597x over previous
"""CFG dual self-attention kernel for 8 Trainium2 NeuronCores.

Strategy (tensor parallel on heads):
  - h = concat(hidden_cond, hidden_uncond) -> [4096 tokens, 5120]; host
    pre-transposes to hT [5120, 4096] so the QKV matmul contraction dim (5120)
    lands on SBUF partitions.
  - Each core owns 5 heads (640 of the 5120 q/k/v channels).  It computes
    qT/kT [640, 4096] (transposed layout: head-dim on partitions) and
    v [4096, 640] (natural layout) from hT with fp32r matmuls.
  - RMSNorm over the full 5120 dims needs a cross-core sum of squares:
    partial ssq per token is computed with ones-matmuls on the PE and
    allreduced across the 8 cores (32 KB collective, hidden under the V
    projection).
  - Attention per (batch, head) in scores-transposed layout
    scoresT[st, sq] = (rope(k) slice)^T @ rope(q): softmax denominators via
    ones-matmul column sums (interleaved PSUM accumulation groups), exp on
    the scalar engine, A@V accumulated with v-chunks stationary, and the
    1/colsum normalization folded into the PSUM->SBUF eviction of attn_outT.
  - Output projection: partial_out = attn_outT^T @ Wout[rows of this core]
    -> [4096, 5120] per-core partial; host sums the 8 partials (+ bout).
"""

import numpy as np
import ml_dtypes

import concourse.bass as bass  # noqa: F401  (bass types via bacc)
import concourse.mybir as mybir
import concourse.tile as tile
from concourse import bacc
from concourse.bass_utils import run_bass_kernel_spmd

F32 = mybir.dt.float32
F32R = mybir.dt.float32r
F8 = mybir.dt.float8e4
E4NP = ml_dtypes.float8_e4m3
DRM = mybir.MatmulPerfMode.DoubleRow

NCORES = 8
EPS = 1e-6

# fp8 hi/lo quantization scales (host-side split; 3-term DoubleRow matmuls)
SH = 16.0
SW = 1024.0
SA = 32.0
SWO = 1024.0
QKV_DESC = 1.0 / (SH * SW)
OUT_DESC = 1.0 / (SA * SWO)


def _vblocks(cw):
    # split the v output width into matmul N-blocks (>=256 keeps fp32r fast)
    if cw % 320 == 0:
        return [320] * (cw // 320)
    return [cw]


def build_program(S, DIM, H, collective=True, repeat=1):
    """Emit the per-core bass program (identical on all cores; per-core data
    differences come entirely from the input tensors)."""
    HD = 128
    assert DIM == H * HD
    HPC = H // NCORES          # heads per core
    CW = HPC * HD              # per-core channel width for q/k/v
    CT = HPC                   # 128-col tiles per group
    NT = 2 * S                 # tokens across both batches
    DC = DIM // 128            # contraction chunks
    TBS = 256                  # token block in phase 1
    NTB = NT // TBS
    SQB = min(512, S)          # sq block in attention
    NSQ = S // SQB
    NST = S // 128             # st (key) chunks per batch
    VNB = _vblocks(CW)
    ONB = DIM // 512           # out-proj N blocks
    # contraction sub-blocks for merged DMA (dep granularity)
    CSUB = 10 if DC % 10 == 0 else DC
    NCS = DC // CSUB

    nc = bacc.Bacc("TRN2", target_bir_lowering=False, debug=False,
                   num_devices=NCORES)

    hT8 = nc.dram_tensor("hT8", [DIM * 2, NT], F8, kind="ExternalInput")
    wq8 = nc.dram_tensor("wq8", [DIM * 2, CW], F8, kind="ExternalInput")
    wk8 = nc.dram_tensor("wk8", [DIM * 2, CW], F8, kind="ExternalInput")
    wv8 = nc.dram_tensor("wv8", [DIM * 2, CW], F8, kind="ExternalInput")
    bq = nc.dram_tensor("bq", [128, CT], F32, kind="ExternalInput")
    bk = nc.dram_tensor("bk", [128, CT], F32, kind="ExternalInput")
    wqn = nc.dram_tensor("wqn", [128, CT], F32, kind="ExternalInput")
    wkn = nc.dram_tensor("wkn", [128, CT], F32, kind="ExternalInput")
    cosT = nc.dram_tensor("cosT", [128, S], F32, kind="ExternalInput")
    sinrT = nc.dram_tensor("sinrT", [128, S], F32, kind="ExternalInput")
    wo8 = nc.dram_tensor("wo8", [6 * 2 * 128, DIM], F8, kind="ExternalInput")
    outp = nc.dram_tensor("outp", [NT, DIM], F32, kind="ExternalOutput")

    h4 = hT8.rearrange("(c j p) t -> p c j t", p=128, j=2)

    with tile.TileContext(nc) as tc:
        with (
            tc.tile_pool(name="dram", bufs=1, space="DRAM") as dram,
            tc.tile_pool(name="persist", bufs=1) as persist,
        ):
            for _rep in range(repeat):
              qsc = dram.tile([CW, NT], F32, tag="qsc")
              ksc = dram.tile([CW, NT], F32, tag="ksc")
              vsc = dram.tile([NT, CW], F32, tag="vsc")
              aosc8 = dram.tile([2 * CW, NT], F8, tag="aosc8")
              cc_in = dram.tile([2, NT], F32, tag="cc_in")
              cc_out = dram.tile([2, NT], F32, tag="cc_out")

              # constants
              ones_f = persist.tile([128, 1], F32, tag="ones_f")
              nc.vector.memset(ones_f[:], 1.0)
              ones = persist.tile([128, 1], F32R, tag="ones")
              nc.vector.tensor_copy(ones[:], ones_f[:])

              bq_t = persist.tile([128, CT], F32, tag="bq")
              nc.sync.dma_start(bq_t[:], bq[:])
              bk_t = persist.tile([128, CT], F32, tag="bk")
              nc.sync.dma_start(bk_t[:], bk[:])
              wqn_t = persist.tile([128, CT], F32, tag="wqn")
              nc.sync.dma_start(wqn_t[:], wqn[:])
              wkn_t = persist.tile([128, CT], F32, tag="wkn")
              nc.sync.dma_start(wkn_t[:], wkn[:])

              # pre-reserved pool for the b=0 norm-factor tiles: opened
              # before the phase-1 pools and closed after phase 3, so their
              # DMAs/broadcasts never WAR-depend on recycled phase-1
              # addresses and can run during the v group
              from contextlib import ExitStack as _ES
              _rb_ctx = _ES()
              rb0p = _rb_ctx.enter_context(tc.tile_pool(name="rb0p", bufs=1))

              # ---------------- phase 1: qkv projections + ssq partials -------
              with (
                  tc.tile_pool(name="wp", bufs=1) as wp,
                  tc.tile_pool(name="hp", bufs=2) as hp,
                  tc.tile_pool(name="ev", bufs=1) as evp,
              ):
                  def load_hall(tb):
                      hall = hp.tile([128, DC, 2, TBS], F8, tag="hall")
                      for cs in range(NCS):
                          nc.sync.dma_start(
                              hall[:, cs * CSUB:(cs + 1) * CSUB, :, :],
                              h4[:, cs * CSUB:(cs + 1) * CSUB, :,
                                 tb * TBS:(tb + 1) * TBS])
                      return hall

                  def load_wall(wdram, interleave_tb0=None, order="ct"):
                      # sub-loads ordered to match the consumer loop: ct-major
                      # for q/k (col-tile outer), chunk-major for v (chunk
                      # outer) — the first matmul group only waits for the
                      # first slice, not the whole W block
                      wall = wp.tile([128, DC, 2, CW], F8, tag="wall")
                      w3 = wdram.rearrange("(c j p) n -> p c j n", p=128,
                                           j=2)
                      hall0 = None
                      if interleave_tb0 is not None:
                          hall0 = hp.tile([128, DC, 2, TBS], F8, tag="hall")
                      if order == "ct":
                          wslices = [(slice(None), slice(ct * 128,
                                                         (ct + 1) * 128))
                                     for ct in range(CT)]
                      else:
                          wslices = [(slice(cs * CSUB, (cs + 1) * CSUB),
                                      slice(None)) for cs in range(NCS)]
                      for si, (csl, nsl) in enumerate(wslices):
                          nc.sync.dma_start(wall[:, csl, :, nsl],
                                            w3[:, csl, :, nsl])
                          if hall0 is not None and si < NCS:
                              nc.sync.dma_start(
                                  hall0[:, si * CSUB:(si + 1) * CSUB, :, :],
                                  h4[:, si * CSUB:(si + 1) * CSUB, :,
                                     0:TBS])
                      if hall0 is not None:
                          for cs in range(len(wslices), NCS):
                              nc.sync.dma_start(
                                  hall0[:, cs * CSUB:(cs + 1) * CSUB, :, :],
                                  h4[:, cs * CSUB:(cs + 1) * CSUB, :,
                                     0:TBS])
                      return wall, hall0

                  for gi, (wdram, bias_t, spill) in enumerate(
                      [(wq8, bq_t, qsc), (wk8, bk_t, ksc)]
                  ):
                      with (
                          tc.tile_pool(name=f"ps{gi}", bufs=4, space="PSUM") as psp,
                          tc.tile_pool(name=f"sq{gi}", bufs=2, space="PSUM") as sqp,
                          tc.tile_pool(name=f"aux{gi}", bufs=1) as st1,
                      ):
                          wall, hall0 = load_wall(wdram, interleave_tb0=True)
                          deferred = []
                          for tb in range(NTB):
                              hall = hall0 if tb == 0 else load_hall(tb)
                              hall0 = None
                              ssq_ps = sqp.tile([1, TBS], F32, tag="ssq")
                              evq = evp.tile([128, CT, TBS], F32R, tag="evq")
                              sqt = st1.tile([128, CT, TBS], F32R, tag="sqt")
                              for ct in range(CT):
                                  pq = psp.tile([128, TBS], F32, tag="acc")
                                  csl = slice(ct * 128, (ct + 1) * 128)
                                  for cp in range(DC // 2):
                                      nc.tensor.matmul(
                                          pq[:],
                                          wall[:, 2 * cp:2 * cp + 2, 0, csl],
                                          hall[:, 2 * cp:2 * cp + 2, 1, :],
                                          start=(cp == 0), stop=False,
                                          perf_mode=DRM)
                                  for ch in range(DC):
                                      nc.tensor.matmul(
                                          pq[:],
                                          wall[:, ch, :, csl],
                                          hall[:, ch, :, :],
                                          start=False, stop=(ch == DC - 1),
                                          perf_mode=DRM)
                                  # the ssq matmul of the PREVIOUS ct group goes
                                  # here so the PE never waits on the DVE square
                                  if deferred:
                                      deferred.pop(0)()
                                  nc.vector.tensor_scalar(
                                      evq[:, ct, :].bitcast(F32), pq[:],
                                      bias_t[:, ct:ct + 1], QKV_DESC,
                                      mybir.AluOpType.add,
                                      mybir.AluOpType.mult)
                                  nc.sync.dma_start(
                                      spill.rearrange("(c p) t -> p c t", p=128)
                                      .bitcast(F32R)
                                      [:, ct, tb * TBS:(tb + 1) * TBS],
                                      evq[:, ct, :])
                                  nc.vector.tensor_mul(
                                      sqt[:, ct, :], evq[:, ct, :].bitcast(F32),
                                      evq[:, ct, :].bitcast(F32))

                                  def emit_ssq(ssq_ps=ssq_ps, sqt=sqt, ct=ct,
                                               evq=evq, tb=tb, spill=spill,
                                               gi=gi):
                                      nc.tensor.matmul(
                                          ssq_ps[:], ones[:], sqt[:, ct, :],
                                          start=(ct == 0), stop=(ct == CT - 1))
                                      if ct == CT - 1:
                                          stg = st1.tile([1, TBS], F32,
                                                         tag="stg", name="stg")
                                          nc.vector.tensor_copy(stg[:],
                                                                ssq_ps[:])
                                          nc.gpsimd.dma_start(
                                              cc_in[gi:gi + 1,
                                                    tb * TBS:(tb + 1) * TBS],
                                              stg[:])
                                  deferred.append(emit_ssq)
                          while deferred:
                              deferred.pop(0)()

                  # allreduce the ssq partials (overlaps with the v group below)
                  if collective:
                      nc.gpsimd.collective_compute(
                          "AllReduce", mybir.AluOpType.add,
                          replica_groups=[list(range(NCORES))],
                          ins=[cc_in[:].opt()], outs=[cc_out[:].opt()])
                  else:
                      # single-core timing-sim variant: stand-in for the
                      # allreduce so TimelineSim (no collectives) can run
                      nc.sync.dma_start(cc_out[:], cc_in[:])

                  # v projection (natural layout, hT token-tiles stationary)
                  with tc.tile_pool(name="psv", bufs=2, space="PSUM") as psv:
                      wall, hall0 = load_wall(wv8, interleave_tb0=True,
                                              order="cs")
                      for tb in range(NTB):
                          hall = hall0 if tb == 0 else load_hall(tb)
                          hall0 = None
                          nsub = TBS // 128
                          pv = [[psv.tile([128, nb], F32, tag=f"pv{ts}_{i}",
                                          name=f"pv{ts}_{i}")
                                 for i, nb in enumerate(VNB)]
                                for ts in range(nsub)]
                          for ts in range(nsub):
                              tsl = slice(ts * 128, (ts + 1) * 128)
                              off = 0
                              for i, nb in enumerate(VNB):
                                  nsl = slice(off, off + nb)
                                  for cp in range(DC // 2):
                                      nc.tensor.matmul(
                                          pv[ts][i][:],
                                          hall[:, 2 * cp:2 * cp + 2, 1, tsl],
                                          wall[:, 2 * cp:2 * cp + 2, 0, nsl],
                                          start=(cp == 0), stop=False,
                                          perf_mode=DRM)
                                  for ch in range(DC):
                                      nc.tensor.matmul(
                                          pv[ts][i][:],
                                          hall[:, ch, :, tsl],
                                          wall[:, ch, :, nsl],
                                          start=False, stop=(ch == DC - 1),
                                          perf_mode=DRM)
                                  off += nb
                          evv = evp.tile([128, nsub, CW], F32R, tag="evq",
                                         name="evv")
                          for ts in range(nsub):
                              off = 0
                              for i, nb in enumerate(VNB):
                                  # v bias is folded into the host-side output
                                  # bias (softmax rows sum to 1), so this is a
                                  # pure PSUM->SBUF copy on the idle scalar
                                  # engine, keeping DVE free for the rinv/rope
                                  # chains that overlap the v group
                                  nc.scalar.activation(
                                      evv[:, ts, off:off + nb].bitcast(F32),
                                      pv[ts][i][:],
                                      mybir.ActivationFunctionType.Copy,
                                      scale=QKV_DESC)
                                  off += nb
                              nc.sync.dma_start(
                                  vsc.rearrange("(b p) n -> p b n", p=128)
                                  .bitcast(F32R)[:, tb * nsub + ts, :],
                                  evv[:, ts, :])

              # ---------------- phase 3: attention per (batch, head) ----------
              # prefix of Wout loaded into its own (non-overlapping) pool while
              # attention still runs, so phase 4 starts without a DMA stall
              WOSL = 1024 if DIM >= 2048 else 512
              w3o = wo8.rearrange("(s j p) n -> p s j n", p=128, j=2)
              with tc.tile_pool(name="wo0p", bufs=1) as wo0p:
                with (
                  tc.tile_pool(name="p3", bufs=2) as p3,
                  tc.tile_pool(name="p3e", bufs=3) as p3e,
                  tc.tile_pool(name="cs3", bufs=1) as cs3,
                  tc.tile_pool(name="ps_sc", bufs=3, space="PSUM") as ps_sc,
                  tc.tile_pool(name="ps_cs", bufs=2, space="PSUM") as ps_cs,
                  tc.tile_pool(name="ps_av", bufs=2, space="PSUM") as ps_av,
                ):
                  cosT_t = cs3.tile([128, S], F32, tag="cosT")
                  nc.sync.dma_start(cosT_t[:], cosT[:])
                  sinrT_t = cs3.tile([128, S], F32, tag="sinrT")
                  nc.sync.dma_start(sinrT_t[:], sinrT[:])
                  wot0 = wo0p.tile([128, 6, 2, WOSL], F8, tag="wot0")

                  # rinv = 1/sqrt(mean ssq + eps): broadcast the allreduced
                  # ssq rows first, then do the math 128-lane-parallel.
                  # Only the b=0 factors are on the critical path; b=1 is
                  # deferred into the first pair's attention.
                  rb = {}

                  def emit_rb(b):
                      for gi in range(2):
                          # fold the HD**-0.5 attention scale into the q side:
                          # s/sqrt(ssq/DIM+eps) == 1/sqrt(ssq*HD/DIM+HD*eps)
                          sc1 = (HD / DIM) if gi == 0 else (1.0 / DIM)
                          sc2 = (HD * EPS) if gi == 0 else EPS
                          row = p3.tile([1, S], F32, tag="rrow", name="rrow")
                          nc.sync.dma_start(
                              row[:], cc_out[gi:gi + 1, b * S:(b + 1) * S])
                          t = cs3.tile([128, S], F32, tag=f"rb{gi}{b}",
                                       name=f"rb{gi}{b}")
                          nc.gpsimd.partition_broadcast(t[:], row[:])
                          nc.vector.tensor_scalar(t[:], t[:], sc1, sc2,
                                                  mybir.AluOpType.mult,
                                                  mybir.AluOpType.add)
                          nc.scalar.activation(
                              t[:], t[:], mybir.ActivationFunctionType.Sqrt)
                          nc.vector.reciprocal(t[:], t[:])
                          rb[(gi, b)] = t

                  emit_rb(0)

                  bhs = [(b, hh) for b in range(2) for hh in range(HPC)]

                  def prep(i, nchunk=1):
                      """Load + norm + rope q/k and load v for pair i.  Emitted
                      one pair ahead so the DVE work overlaps the previous
                      pair's attention matmuls.  nchunk>1 slices the chain
                      column-wise so the first attention matmul only waits
                      for the first slice (used for the first pair, whose
                      chain cannot overlap anything earlier)."""
                      b, hh = bhs[i]
                      CS2 = S // nchunk
                      qkr = []
                      for gi, (spill, wn) in enumerate(
                              [(qsc, wqn_t), (ksc, wkn_t)]):
                          xt = p3.tile([128, S], F32, tag="xt", name="xt")
                          tmc = p3.tile([128, S], F32, tag="tmc", name="tmc")
                          tms = p3.tile([128, S], F32, tag="tms", name="tms")
                          xr = p3.tile([128, S], F32R, tag="xr", name="xr")
                          for cc in range(nchunk):
                              sl = slice(cc * CS2, (cc + 1) * CS2)
                              nc.sync.dma_start(
                                  xt[:, sl],
                                  spill[hh * 128:(hh + 1) * 128,
                                        b * S + cc * CS2:
                                        b * S + (cc + 1) * CS2])
                              nc.vector.tensor_mul(xt[:, sl], xt[:, sl],
                                                   rb[(gi, b)][:, sl])
                              nc.vector.tensor_scalar_mul(
                                  xt[:, sl], xt[:, sl], wn[:, hh:hh + 1])
                              nc.vector.tensor_mul(tmc[:, sl], xt[:, sl],
                                                   cosT_t[:, sl])
                              nc.vector.tensor_mul(
                                  tms[0:64, sl], xt[64:128, sl],
                                  sinrT_t[64:128, sl])
                              nc.vector.tensor_mul(
                                  tms[64:128, sl], xt[0:64, sl],
                                  sinrT_t[0:64, sl])
                              nc.vector.tensor_add(xr[:, sl], tmc[:, sl],
                                                   tms[:, sl])
                          qkr.append(xr)
                      vt = p3.tile([128, NST, 128], F32R, tag="vt", name="vt")
                      nc.gpsimd.dma_start(
                          vt[:], vsc[b * S:(b + 1) * S,
                                     hh * 128:(hh + 1) * 128]
                          .bitcast(F32R)
                          .rearrange("(c p) d -> p c d", p=128))
                      return qkr[0], qkr[1], vt

                  preps = {0: prep(0, nchunk=4)}
                  for i in range(len(bhs)):
                      b, hh = bhs[i]
                      if i + 1 < len(bhs) and i > 0:
                          preps[i + 1] = prep(i + 1)
                      qr, kr, vt = preps.pop(i)
                      if True:
                          aoh8 = p3.tile([128, S], F8, tag="aoh8",
                                         name="aoh8")
                          aol8 = p3.tile([128, S], F8, tag="aol8",
                                         name="aol8")
                          for sqb in range(NSQ):
                              cs = ps_cs.tile([1, SQB], F32, tag="cs")
                              av = ps_av.tile([128, SQB], F32, tag="av")
                              # one-step lookahead: scores MM for st+1 issues
                              # before the cs/av MMs of st, hiding exp latency
                              ets = {}
                              for st in range(NST + 1):
                                  if st < NST:
                                      sc = ps_sc.tile([128, SQB], F32, tag="sc")
                                      nc.tensor.matmul(
                                          sc[:], kr[:, st * 128:(st + 1) * 128],
                                          qr[:, sqb * SQB:(sqb + 1) * SQB],
                                          start=True, stop=True)
                                      et = p3e.tile([128, SQB], F32R, tag="et")
                                      nc.scalar.activation(
                                          et[:], sc[:],
                                          mybir.ActivationFunctionType.Exp)
                                      ets[st] = et
                                  if st >= 1:
                                      pst = st - 1
                                      et = ets.pop(pst)
                                      nc.tensor.matmul(
                                          cs[:], ones[:], et[:],
                                          start=(pst == 0),
                                          stop=(pst == NST - 1))
                                      nc.tensor.matmul(
                                          av[:], vt[:, pst, :], et[:],
                                          start=(pst == 0),
                                          stop=(pst == NST - 1))
                              rc = p3.tile([1, SQB], F32, tag="rc")
                              nc.vector.reciprocal(rc[:], cs[:])
                              nc.vector.tensor_scalar_mul(rc[:], rc[:], SA)
                              rb2 = p3.tile([128, SQB], F32, tag="rb2")
                              nc.gpsimd.partition_broadcast(rb2[:], rc[:])
                              sqsl = slice(sqb * SQB, (sqb + 1) * SQB)
                              ao32 = p3.tile([128, SQB], F32, tag="ao32")
                              nc.vector.tensor_mul(ao32[:], av[:], rb2[:])
                              nc.vector.tensor_copy(aoh8[:, sqsl], ao32[:])
                              nc.vector.tensor_tensor(
                                  aol8[:, sqsl], ao32[:], aoh8[:, sqsl],
                                  mybir.AluOpType.subtract)
                              if i == 0 and sqb == 0:
                                  # off the critical path: the b=1 norm
                                  # factors and the second pair's prep
                                  emit_rb(1)
                                  preps[1] = prep(1)
                                  # prefetch of the Wout prefix for phase 4
                                  for nb in range(WOSL // 512):
                                      nc.sync.dma_start(
                                          wot0[:, :, :,
                                               nb * 512:(nb + 1) * 512],
                                          w3o[:, :, :,
                                              nb * 512:(nb + 1) * 512])
                          nc.gpsimd.dma_start(
                              aosc8[hh * 256 + 128:hh * 256 + 256,
                                    b * S:(b + 1) * S], aoh8[:])
                          nc.gpsimd.dma_start(
                              aosc8[hh * 256:hh * 256 + 128,
                                    b * S:(b + 1) * S], aol8[:])

                # -------------- phase 4: partial output projection ----------
                NWO0 = WOSL // 512
                with (
                  tc.tile_pool(name="wo", bufs=1) as wo,
                  tc.tile_pool(name="p4", bufs=4) as p4,
                  tc.tile_pool(name="oe", bufs=2) as oep,
                  tc.tile_pool(name="ps4", bufs=4, space="PSUM") as ps4,
                ):
                  ao4 = aosc8.rearrange("(c j p) t -> p c j t", p=128,
                                        j=2)

                  def load_aot(tt):
                      aot = p4.tile([128, HPC, 2, 128], F8, tag="aot",
                                    name="aot")
                      nc.sync.dma_start(
                          aot[:], ao4[:, :, :, tt * 128:(tt + 1) * 128])
                      return aot

                  aot_next = load_aot(0)   # first attn-out tile before W bulk
                  wot = None
                  if DIM > WOSL:
                      wot = wo.tile([128, 6, 2, DIM - WOSL], F8, tag="wot")
                      for nb in range(NWO0, ONB):
                          nc.sync.dma_start(
                              wot[:, :, :, (nb - NWO0) * 512:
                                  (nb - NWO0 + 1) * 512],
                              w3o[:, :, :, nb * 512:(nb + 1) * 512])
                  for tt in range(NT // 128):
                      aot = aot_next
                      if tt + 1 < NT // 128:
                          aot_next = load_aot(tt + 1)
                      orow = oep.tile([128, DIM], F32, tag="orow")
                      for nb in range(ONB):
                          if nb < NWO0:
                              wsl = wot0[:, :, :, nb * 512:(nb + 1) * 512]
                          else:
                              wsl = wot[:, :, :, (nb - NWO0) * 512:
                                        (nb - NWO0 + 1) * 512]
                          po = ps4.tile([128, 512], F32, tag="po")
                          # 8 DoubleRow passes: 2 hi*hi chunk-pairs, 5
                          # crosses, 1 fused hi*hi+lo*lo for the odd chunk
                          nc.tensor.matmul(
                              po[:], aot[:, 0:2, 1, :], wsl[:, 0:2, 0, :],
                              start=True, stop=False, perf_mode=DRM)
                          nc.tensor.matmul(
                              po[:], aot[:, 2:4, 1, :], wsl[:, 2:4, 0, :],
                              start=False, stop=False, perf_mode=DRM)
                          for ch in range(HPC):
                              nc.tensor.matmul(
                                  po[:], aot[:, ch, :, :], wsl[:, ch, :, :],
                                  start=False, stop=False, perf_mode=DRM)
                          nc.tensor.matmul(
                              po[:], aot[:, 4, :, :], wsl[:, 5, :, :],
                              start=False, stop=True, perf_mode=DRM)
                          nc.vector.tensor_scalar_mul(
                              orow[:, nb * 512:(nb + 1) * 512], po[:],
                              OUT_DESC)
                      nc.sync.dma_start(
                          outp[tt * 128:(tt + 1) * 128, :], orow[:])
              _rb_ctx.close()
    nc.finalize()
    return nc


_PROGRAM_CACHE = {}


def _get_program(S, DIM, H):
    key = (S, DIM, H)
    if key not in _PROGRAM_CACHE:
        _PROGRAM_CACHE[key] = build_program(S, DIM, H)
    return _PROGRAM_CACHE[key]


def _split8(x, scale):
    xs = (np.asarray(x, np.float32) * np.float32(scale))
    hi = xs.astype(E4NP)
    lo = (xs - hi.astype(np.float32)).astype(E4NP)
    return hi, lo


def _pack_w(Wslice, scale):
    # [DIM, n] -> [(c j p), n] with j=(hi, lo)
    wh, wl = _split8(Wslice, scale)
    d, n = Wslice.shape
    w8 = np.empty((d // 128, 2, 128, n), E4NP)
    w8[:, 0] = wh.reshape(d // 128, 128, n)
    w8[:, 1] = wl.reshape(d // 128, 128, n)
    return np.ascontiguousarray(w8.reshape(d * 2, n))


def make_in_maps(S, DIM, H, hidden_cond, hidden_uncond, cos_freqs, sin_freqs,
                 Wqkv, bqkv, wq_norm, wk_norm, Wout, bout):
    HD = 128
    HPC = H // NCORES
    CW = HPC * HD
    NT = 2 * S
    h = np.concatenate([np.asarray(hidden_cond), np.asarray(hidden_uncond)],
                       axis=0).reshape(NT, DIM)
    hT = np.ascontiguousarray(h.T)
    hh, hl = _split8(hT, SH)
    hT8 = np.empty((DIM // 128, 2, 128, NT), E4NP)
    hT8[:, 0] = hl.reshape(DIM // 128, 128, NT)   # j=0: lo
    hT8[:, 1] = hh.reshape(DIM // 128, 128, NT)   # j=1: hi
    hT8 = np.ascontiguousarray(hT8.reshape(DIM * 2, NT))
    cosT = np.ascontiguousarray(np.asarray(cos_freqs).T.astype(np.float32))
    sinT = np.asarray(sin_freqs).T  # [128, S]
    HF = HD // 2
    sinrT = np.concatenate([sinT[HF:], -sinT[:HF]], axis=0)
    sinrT = np.ascontiguousarray(sinrT.astype(np.float32))
    Wqkv = np.asarray(Wqkv)
    bqkv = np.asarray(bqkv)
    wq_norm = np.asarray(wq_norm)
    wk_norm = np.asarray(wk_norm)
    Wout = np.asarray(Wout)

    in_maps = []
    for c in range(NCORES):
        sl = slice(c * CW, (c + 1) * CW)
        bq_c = (bqkv[0 * DIM:1 * DIM][sl] * (SH * SW)).astype(np.float32) \
            .reshape(HPC, HD).T
        bk_c = (bqkv[1 * DIM:2 * DIM][sl] * (SH * SW)).astype(np.float32) \
            .reshape(HPC, HD).T
        # out-proj rows (slot, j, p): slots 0..4 = chunk (hi,lo), slot 5 =
        # chunk 4 as (lo,hi) so its hi*hi+lo*lo fuses into one DoubleRow
        woh, wol = _split8(Wout[sl, :], SWO)
        wo8 = np.empty((6, 2, 128, DIM), E4NP)
        for s in range(HPC):
            wo8[s, 0] = woh[s * 128:(s + 1) * 128]
            wo8[s, 1] = wol[s * 128:(s + 1) * 128]
        wo8[5, 0] = wol[4 * 128:5 * 128]
        wo8[5, 1] = woh[4 * 128:5 * 128]
        in_maps.append({
            "hT8": hT8,
            "wq8": _pack_w(Wqkv[:, 0 * DIM:1 * DIM][:, sl], SW),
            "wk8": _pack_w(Wqkv[:, 1 * DIM:2 * DIM][:, sl], SW),
            "wv8": _pack_w(Wqkv[:, 2 * DIM:3 * DIM][:, sl], SW),
            "bq": np.ascontiguousarray(bq_c),
            "bk": np.ascontiguousarray(bk_c),
            "wqn": np.ascontiguousarray(wq_norm[sl].reshape(HPC, HD).T
                                        .astype(np.float32)),
            "wkn": np.ascontiguousarray(wk_norm[sl].reshape(HPC, HD).T
                                        .astype(np.float32)),
            "cosT": cosT,
            "sinrT": sinrT,
            "wo8": np.ascontiguousarray(wo8.reshape(6 * 2 * 128, DIM)),
        })
    return in_maps


def run(S, DIM, H, inputs):
    nc = _get_program(S, DIM, H)
    in_maps = make_in_maps(S, DIM, H, **inputs)
    res = run_bass_kernel_spmd(nc, in_maps, list(range(NCORES)))
    partial = np.zeros((2 * S, DIM), np.float64)
    for r in res.results:
        partial += r["outp"].astype(np.float64)
    # the v-bias contribution: softmax rows sum to 1, so attn(v + 1*bv) =
    # attn(v) + 1*bv, and bv flows through Wout as a constant per-channel term
    bv_full = np.asarray(inputs["bqkv"])[2 * DIM:3 * DIM].astype(np.float64)
    const_bias = bv_full @ np.asarray(inputs["Wout"]).astype(np.float64) \
        + np.asarray(inputs["bout"])
    out = (partial + const_bias[None, :]).astype(np.float32)
    out = out.reshape(2, 1, S, DIM)
    return out[0], out[1]


def kernel(hidden_cond, hidden_uncond, cos_freqs, sin_freqs,
           Wqkv, bqkv, wq_norm, wk_norm, Wout, bout):
    B, S, DIM = np.asarray(hidden_cond).shape
    assert B == 1
    H = DIM // 128
    return run(S, DIM, H, dict(
        hidden_cond=hidden_cond, hidden_uncond=hidden_uncond,
        cos_freqs=cos_freqs, sin_freqs=sin_freqs, Wqkv=Wqkv, bqkv=bqkv,
        wq_norm=wq_norm, wk_norm=wk_norm, Wout=Wout, bout=bout))



# revision 28
# speedup vs baseline: 1.1792x; 1.0168x over previous
"""CFG dual self-attention kernel for 8 Trainium2 NeuronCores.

Strategy (tensor parallel on heads):
  - h = concat(hidden_cond, hidden_uncond) -> [4096 tokens, 5120]; host
    pre-transposes to hT [5120, 4096] so the QKV matmul contraction dim (5120)
    lands on SBUF partitions.
  - Each core owns 5 heads (640 of the 5120 q/k/v channels).  It computes
    qT/kT [640, 4096] (transposed layout: head-dim on partitions) and
    v [4096, 640] (natural layout) from hT with fp32r matmuls.
  - RMSNorm over the full 5120 dims needs a cross-core sum of squares:
    partial ssq per token is computed with ones-matmuls on the PE and
    allreduced across the 8 cores (32 KB collective, hidden under the V
    projection).
  - Attention per (batch, head) in scores-transposed layout
    scoresT[st, sq] = (rope(k) slice)^T @ rope(q): softmax denominators via
    ones-matmul column sums (interleaved PSUM accumulation groups), exp on
    the scalar engine, A@V accumulated with v-chunks stationary, and the
    1/colsum normalization folded into the PSUM->SBUF eviction of attn_outT.
  - Output projection: partial_out = attn_outT^T @ Wout[rows of this core]
    -> [4096, 5120] per-core partial; host sums the 8 partials (+ bout).
"""

import numpy as np
import ml_dtypes

import concourse.bass as bass  # noqa: F401  (bass types via bacc)
import concourse.mybir as mybir
import concourse.tile as tile
from concourse import bacc
from concourse.bass_utils import run_bass_kernel_spmd

F32 = mybir.dt.float32
F32R = mybir.dt.float32r
BF16 = mybir.dt.bfloat16
F8 = mybir.dt.float8e4
E4NP = ml_dtypes.float8_e4m3
DRM = mybir.MatmulPerfMode.DoubleRow

NCORES = 8
EPS = 1e-6

# fp8 hi/lo quantization scales (host-side split; 3-term DoubleRow matmuls)
SH = 16.0
SW = 1024.0
SA = 32.0
SWO = 1024.0
QKV_DESC = 1.0 / (SH * SW)
OUT_DESC = 1.0 / (SA * SWO)


def _vblocks(cw):
    # split the v output width into matmul N-blocks (>=256 keeps fp32r fast)
    if cw % 320 == 0:
        return [320] * (cw // 320)
    return [cw]


def build_program(S, DIM, H, collective=True, repeat=1):
    """Emit the per-core bass program (identical on all cores; per-core data
    differences come entirely from the input tensors)."""
    HD = 128
    assert DIM == H * HD
    HPC = H // NCORES          # heads per core
    CW = HPC * HD              # per-core channel width for q/k/v
    CT = HPC                   # 128-col tiles per group
    NT = 2 * S                 # tokens across both batches
    DC = DIM // 128            # contraction chunks
    TBS = 256                  # token block in phase 1
    NTB = NT // TBS
    SQB = min(512, S)          # sq block in attention
    NSQ = S // SQB
    NST = S // 128             # st (key) chunks per batch
    VNB = _vblocks(CW)
    ONB = DIM // 512           # out-proj N blocks
    # contraction sub-blocks for merged DMA (dep granularity)
    CSUB = 10 if DC % 10 == 0 else DC
    NCS = DC // CSUB

    nc = bacc.Bacc("TRN2", target_bir_lowering=False, debug=False,
                   num_devices=NCORES)

    hT8 = nc.dram_tensor("hT8", [DIM * 2, NT], F8, kind="ExternalInput")
    wq8 = nc.dram_tensor("wq8", [DIM * 2, CW], F8, kind="ExternalInput")
    wk8 = nc.dram_tensor("wk8", [DIM * 2, CW], F8, kind="ExternalInput")
    wv8 = nc.dram_tensor("wv8", [DIM * 2, CW], F8, kind="ExternalInput")
    bq = nc.dram_tensor("bq", [128, CT], F32, kind="ExternalInput")
    bk = nc.dram_tensor("bk", [128, CT], F32, kind="ExternalInput")
    wqn = nc.dram_tensor("wqn", [128, CT], F32, kind="ExternalInput")
    wkn = nc.dram_tensor("wkn", [128, CT], F32, kind="ExternalInput")
    cosT = nc.dram_tensor("cosT", [128, S], BF16, kind="ExternalInput")
    sinrT = nc.dram_tensor("sinrT", [128, S], BF16, kind="ExternalInput")
    wo8 = nc.dram_tensor("wo8", [6 * 2 * 128, DIM], F8, kind="ExternalInput")
    outp = nc.dram_tensor("outp", [NT, DIM], F32, kind="ExternalOutput")

    h4 = hT8.rearrange("(c j p) t -> p c j t", p=128, j=2)

    with tile.TileContext(nc) as tc:
        with (
            tc.tile_pool(name="dram", bufs=1, space="DRAM") as dram,
            tc.tile_pool(name="persist", bufs=1) as persist,
        ):
            for _rep in range(repeat):
              qsc = dram.tile([CW, NT], BF16, tag="qsc")
              ksc = dram.tile([CW, NT], BF16, tag="ksc")
              vsc = dram.tile([NT, CW], F32, tag="vsc")
              aosc8 = dram.tile([2 * CW, NT], F8, tag="aosc8")
              cc_in = dram.tile([2, NT], F32, tag="cc_in")
              cc_out = dram.tile([2, NT], F32, tag="cc_out")

              # constants
              ones_f = persist.tile([128, 1], F32, tag="ones_f")
              nc.vector.memset(ones_f[:], 1.0)
              ones = persist.tile([128, 1], F32R, tag="ones")
              nc.vector.tensor_copy(ones[:], ones_f[:])

              bq_t = persist.tile([128, CT], F32, tag="bq")
              nc.sync.dma_start(bq_t[:], bq[:])
              bk_t = persist.tile([128, CT], F32, tag="bk")
              nc.sync.dma_start(bk_t[:], bk[:])
              wqn_t = persist.tile([128, CT], F32, tag="wqn")
              nc.sync.dma_start(wqn_t[:], wqn[:])
              wkn_t = persist.tile([128, CT], F32, tag="wkn")
              nc.sync.dma_start(wkn_t[:], wkn[:])

              # pre-reserved pool for the b=0 norm-factor tiles: opened
              # before the phase-1 pools and closed after phase 3, so their
              # DMAs/broadcasts never WAR-depend on recycled phase-1
              # addresses and can run during the v group
              from contextlib import ExitStack as _ES
              _rb_ctx = _ES()
              rb0p = _rb_ctx.enter_context(tc.tile_pool(name="rb0p", bufs=1))

              # ---------------- phase 1: qkv projections + ssq partials -------
              with (
                  tc.tile_pool(name="wp", bufs=1) as wp,
                  tc.tile_pool(name="hp", bufs=2) as hp,
                  tc.tile_pool(name="ev", bufs=1) as evp,
              ):
                  def load_hall(tb):
                      hall = hp.tile([128, DC, 2, TBS], F8, tag="hall")
                      for cs in range(NCS):
                          nc.sync.dma_start(
                              hall[:, cs * CSUB:(cs + 1) * CSUB, :, :],
                              h4[:, cs * CSUB:(cs + 1) * CSUB, :,
                                 tb * TBS:(tb + 1) * TBS])
                      return hall

                  def load_wall(wdram, interleave_tb0=None, order="ct"):
                      # sub-loads ordered to match the consumer loop: ct-major
                      # for q/k (col-tile outer), chunk-major for v (chunk
                      # outer) — the first matmul group only waits for the
                      # first slice, not the whole W block
                      wall = wp.tile([128, DC, 2, CW], F8, tag="wall")
                      w3 = wdram.rearrange("(c j p) n -> p c j n", p=128,
                                           j=2)
                      hall0 = None
                      if interleave_tb0 is not None:
                          hall0 = hp.tile([128, DC, 2, TBS], F8, tag="hall")
                      if order == "ct":
                          wslices = [(slice(None), slice(ct * 128,
                                                         (ct + 1) * 128))
                                     for ct in range(CT)]
                      else:
                          wslices = [(slice(cs * CSUB, (cs + 1) * CSUB),
                                      slice(None)) for cs in range(NCS)]
                      for si, (csl, nsl) in enumerate(wslices):
                          nc.sync.dma_start(wall[:, csl, :, nsl],
                                            w3[:, csl, :, nsl])
                          if hall0 is not None and si < NCS:
                              nc.sync.dma_start(
                                  hall0[:, si * CSUB:(si + 1) * CSUB, :, :],
                                  h4[:, si * CSUB:(si + 1) * CSUB, :,
                                     0:TBS])
                      if hall0 is not None:
                          for cs in range(len(wslices), NCS):
                              nc.sync.dma_start(
                                  hall0[:, cs * CSUB:(cs + 1) * CSUB, :, :],
                                  h4[:, cs * CSUB:(cs + 1) * CSUB, :,
                                     0:TBS])
                      return wall, hall0

                  for gi, (wdram, bias_t, spill) in enumerate(
                      [(wq8, bq_t, qsc), (wk8, bk_t, ksc)]
                  ):
                      with (
                          tc.tile_pool(name=f"ps{gi}", bufs=4, space="PSUM") as psp,
                          tc.tile_pool(name=f"sq{gi}", bufs=2, space="PSUM") as sqp,
                          tc.tile_pool(name=f"aux{gi}", bufs=1) as st1,
                      ):
                          wall, hall0 = load_wall(wdram, interleave_tb0=True)
                          deferred = []
                          for tb in range(NTB):
                              hall = hall0 if tb == 0 else load_hall(tb)
                              hall0 = None
                              ssq_ps = sqp.tile([1, TBS], F32, tag="ssq")
                              evq = evp.tile([128, CT, TBS], BF16, tag="evq")
                              sqt = st1.tile([128, CT, TBS], F32R, tag="sqt")
                              for ct in range(CT):
                                  pq = psp.tile([128, TBS], F32, tag="acc")
                                  csl = slice(ct * 128, (ct + 1) * 128)
                                  for cp in range(DC // 2):
                                      nc.tensor.matmul(
                                          pq[:],
                                          wall[:, 2 * cp:2 * cp + 2, 0, csl],
                                          hall[:, 2 * cp:2 * cp + 2, 1, :],
                                          start=(cp == 0), stop=False,
                                          perf_mode=DRM)
                                  for ch in range(DC):
                                      nc.tensor.matmul(
                                          pq[:],
                                          wall[:, ch, :, csl],
                                          hall[:, ch, :, :],
                                          start=False, stop=(ch == DC - 1),
                                          perf_mode=DRM)
                                  # the ssq matmul of the PREVIOUS ct group goes
                                  # here so the PE never waits on the DVE square
                                  if deferred:
                                      deferred.pop(0)()
                                  nc.vector.tensor_scalar(
                                      evq[:, ct, :], pq[:],
                                      bias_t[:, ct:ct + 1], QKV_DESC,
                                      mybir.AluOpType.add,
                                      mybir.AluOpType.mult)
                                  nc.sync.dma_start(
                                      spill.rearrange("(c p) t -> p c t",
                                                      p=128)
                                      [:, ct, tb * TBS:(tb + 1) * TBS],
                                      evq[:, ct, :])
                                  nc.vector.tensor_mul(
                                      sqt[:, ct, :].bitcast(F32),
                                      evq[:, ct, :], evq[:, ct, :])

                                  def emit_ssq(ssq_ps=ssq_ps, sqt=sqt, ct=ct,
                                               evq=evq, tb=tb, spill=spill,
                                               gi=gi):
                                      nc.tensor.matmul(
                                          ssq_ps[:], ones[:], sqt[:, ct, :],
                                          start=(ct == 0), stop=(ct == CT - 1))
                                      if ct == CT - 1:
                                          stg = st1.tile([1, TBS], F32,
                                                         tag="stg", name="stg")
                                          nc.vector.tensor_copy(stg[:],
                                                                ssq_ps[:])
                                          nc.gpsimd.dma_start(
                                              cc_in[gi:gi + 1,
                                                    tb * TBS:(tb + 1) * TBS],
                                              stg[:])
                                  deferred.append(emit_ssq)
                          while deferred:
                              deferred.pop(0)()

                  # allreduce the ssq partials (overlaps with the v group below)
                  if collective:
                      nc.gpsimd.collective_compute(
                          "AllReduce", mybir.AluOpType.add,
                          replica_groups=[list(range(NCORES))],
                          ins=[cc_in[:].opt()], outs=[cc_out[:].opt()])
                  else:
                      # single-core timing-sim variant: stand-in for the
                      # allreduce so TimelineSim (no collectives) can run
                      nc.sync.dma_start(cc_out[:], cc_in[:])

                  # v projection (natural layout, hT token-tiles stationary)
                  with tc.tile_pool(name="psv", bufs=2, space="PSUM") as psv:
                      wall, hall0 = load_wall(wv8, interleave_tb0=True,
                                              order="cs")
                      for tb in range(NTB):
                          hall = hall0 if tb == 0 else load_hall(tb)
                          hall0 = None
                          nsub = TBS // 128
                          pv = [[psv.tile([128, nb], F32, tag=f"pv{ts}_{i}",
                                          name=f"pv{ts}_{i}")
                                 for i, nb in enumerate(VNB)]
                                for ts in range(nsub)]
                          for ts in range(nsub):
                              tsl = slice(ts * 128, (ts + 1) * 128)
                              off = 0
                              for i, nb in enumerate(VNB):
                                  nsl = slice(off, off + nb)
                                  for cp in range(DC // 2):
                                      nc.tensor.matmul(
                                          pv[ts][i][:],
                                          hall[:, 2 * cp:2 * cp + 2, 1, tsl],
                                          wall[:, 2 * cp:2 * cp + 2, 0, nsl],
                                          start=(cp == 0), stop=False,
                                          perf_mode=DRM)
                                  for ch in range(DC):
                                      nc.tensor.matmul(
                                          pv[ts][i][:],
                                          hall[:, ch, :, tsl],
                                          wall[:, ch, :, nsl],
                                          start=False, stop=(ch == DC - 1),
                                          perf_mode=DRM)
                                  off += nb
                          evv = evp.tile([128, nsub, CW], F32R, tag="evq",
                                         name="evv")
                          for ts in range(nsub):
                              off = 0
                              for i, nb in enumerate(VNB):
                                  # v bias is folded into the host-side output
                                  # bias (softmax rows sum to 1), so this is a
                                  # pure PSUM->SBUF copy on the idle scalar
                                  # engine, keeping DVE free for the rinv/rope
                                  # chains that overlap the v group
                                  nc.scalar.activation(
                                      evv[:, ts, off:off + nb].bitcast(F32),
                                      pv[ts][i][:],
                                      mybir.ActivationFunctionType.Copy,
                                      scale=QKV_DESC)
                                  off += nb
                              nc.sync.dma_start(
                                  vsc.rearrange("(b p) n -> p b n", p=128)
                                  .bitcast(F32R)[:, tb * nsub + ts, :],
                                  evv[:, ts, :])

              # ---------------- phase 3: attention per (batch, head) ----------
              # prefix of Wout loaded into its own (non-overlapping) pool while
              # attention still runs, so phase 4 starts without a DMA stall
              WOSL = 1024 if DIM >= 2048 else 512
              w3o = wo8.rearrange("(s j p) n -> p s j n", p=128, j=2)
              with tc.tile_pool(name="wo0p", bufs=1) as wo0p:
                with (
                  tc.tile_pool(name="p3", bufs=2) as p3,
                  tc.tile_pool(name="p3e", bufs=3) as p3e,
                  tc.tile_pool(name="cs3", bufs=1) as cs3,
                  tc.tile_pool(name="ps_sc", bufs=3, space="PSUM") as ps_sc,
                  tc.tile_pool(name="ps_cs", bufs=2, space="PSUM") as ps_cs,
                  tc.tile_pool(name="ps_av", bufs=2, space="PSUM") as ps_av,
                ):
                  cosT_t = cs3.tile([128, S], BF16, tag="cosT")
                  nc.sync.dma_start(cosT_t[:], cosT[:])
                  sinrT_t = cs3.tile([128, S], BF16, tag="sinrT")
                  nc.sync.dma_start(sinrT_t[:], sinrT[:])
                  wot0 = wo0p.tile([128, 6, 2, WOSL], F8, tag="wot0")

                  # rinv = 1/sqrt(mean ssq + eps): broadcast the allreduced
                  # ssq rows first, then do the math 128-lane-parallel.
                  # Only the b=0 factors are on the critical path; b=1 is
                  # deferred into the first pair's attention.
                  rb = {}

                  def emit_rb(b):
                      for gi in range(2):
                          # fold the HD**-0.5 attention scale into the q side:
                          # s/sqrt(ssq/DIM+eps) == 1/sqrt(ssq*HD/DIM+HD*eps)
                          sc1 = (HD / DIM) if gi == 0 else (1.0 / DIM)
                          sc2 = (HD * EPS) if gi == 0 else EPS
                          row = p3.tile([1, S], F32, tag="rrow", name="rrow")
                          nc.sync.dma_start(
                              row[:], cc_out[gi:gi + 1, b * S:(b + 1) * S])
                          t = cs3.tile([128, S], F32, tag=f"rb{gi}{b}",
                                       name=f"rb{gi}{b}")
                          nc.gpsimd.partition_broadcast(t[:], row[:])
                          nc.vector.tensor_scalar(t[:], t[:], sc1, sc2,
                                                  mybir.AluOpType.mult,
                                                  mybir.AluOpType.add)
                          nc.scalar.activation(
                              t[:], t[:], mybir.ActivationFunctionType.Sqrt)
                          tb16 = cs3.tile([128, S], BF16, tag=f"rc{gi}{b}",
                                          name=f"rc{gi}{b}")
                          with nc.allow_low_precision(
                                  reason="rinv factors are fine in bf16"):
                              nc.vector.reciprocal(tb16[:], t[:])
                          rb[(gi, b)] = tb16

                  emit_rb(0)

                  bhs = [(b, hh) for b in range(2) for hh in range(HPC)]

                  def prep(i, nchunk=1):
                      """Load + norm + rope q/k and load v for pair i.  Emitted
                      one pair ahead so the DVE work overlaps the previous
                      pair's attention matmuls.  nchunk>1 slices the chain
                      column-wise so the first attention matmul only waits
                      for the first slice (used for the first pair, whose
                      chain cannot overlap anything earlier)."""
                      b, hh = bhs[i]
                      CS2 = S // nchunk
                      qkr = []
                      for gi, (spill, wn) in enumerate(
                              [(qsc, wqn_t), (ksc, wkn_t)]):
                          xt = p3.tile([128, S], BF16, tag="xt", name="xt")
                          tmc = p3.tile([128, S], BF16, tag="tmc",
                                        name="tmc")
                          tms = p3.tile([128, S], BF16, tag="tms",
                                        name="tms")
                          xr = p3.tile([128, S], BF16, tag="xr", name="xr")
                          for cc in range(nchunk):
                              sl = slice(cc * CS2, (cc + 1) * CS2)
                              nc.sync.dma_start(
                                  xt[:, sl],
                                  spill[hh * 128:(hh + 1) * 128,
                                        b * S + cc * CS2:
                                        b * S + (cc + 1) * CS2])
                              nc.vector.tensor_mul(xt[:, sl], xt[:, sl],
                                                   rb[(gi, b)][:, sl])
                              nc.vector.tensor_scalar_mul(
                                  xt[:, sl], xt[:, sl], wn[:, hh:hh + 1])
                              nc.vector.tensor_mul(tmc[:, sl], xt[:, sl],
                                                   cosT_t[:, sl])
                              nc.vector.tensor_mul(
                                  tms[0:64, sl], xt[64:128, sl],
                                  sinrT_t[64:128, sl])
                              nc.vector.tensor_mul(
                                  tms[64:128, sl], xt[0:64, sl],
                                  sinrT_t[0:64, sl])
                              nc.vector.tensor_add(xr[:, sl], tmc[:, sl],
                                                   tms[:, sl])
                          qkr.append(xr)
                      vt = p3.tile([128, NST, 128], F32R, tag="vt", name="vt")
                      nc.gpsimd.dma_start(
                          vt[:], vsc[b * S:(b + 1) * S,
                                     hh * 128:(hh + 1) * 128]
                          .bitcast(F32R)
                          .rearrange("(c p) d -> p c d", p=128))
                      return qkr[0], qkr[1], vt

                  preps = {0: prep(0, nchunk=4)}
                  for i in range(len(bhs)):
                      b, hh = bhs[i]
                      if i + 1 < len(bhs) and i > 0:
                          preps[i + 1] = prep(i + 1)
                      qr, kr, vt = preps.pop(i)
                      if True:
                          aoh8 = p3.tile([128, S], F8, tag="aoh8",
                                         name="aoh8")
                          aol8 = p3.tile([128, S], F8, tag="aol8",
                                         name="aol8")
                          for sqb in range(NSQ):
                              cs = ps_cs.tile([1, SQB], F32, tag="cs")
                              av = ps_av.tile([128, SQB], F32, tag="av")
                              # one-step lookahead: scores MM for st+1 issues
                              # before the cs/av MMs of st, hiding exp latency
                              ets = {}
                              for st in range(NST + 1):
                                  if st < NST:
                                      sc = ps_sc.tile([128, SQB], F32, tag="sc")
                                      nc.tensor.matmul(
                                          sc[:], kr[:, st * 128:(st + 1) * 128],
                                          qr[:, sqb * SQB:(sqb + 1) * SQB],
                                          start=True, stop=True)
                                      et = p3e.tile([128, SQB], F32R, tag="et")
                                      nc.scalar.activation(
                                          et[:], sc[:],
                                          mybir.ActivationFunctionType.Exp)
                                      ets[st] = et
                                  if st >= 1:
                                      pst = st - 1
                                      et = ets.pop(pst)
                                      nc.tensor.matmul(
                                          cs[:], ones[:], et[:],
                                          start=(pst == 0),
                                          stop=(pst == NST - 1))
                                      nc.tensor.matmul(
                                          av[:], vt[:, pst, :], et[:],
                                          start=(pst == 0),
                                          stop=(pst == NST - 1))
                              rc = p3.tile([1, SQB], F32, tag="rc")
                              nc.vector.reciprocal(rc[:], cs[:])
                              nc.vector.tensor_scalar_mul(rc[:], rc[:], SA)
                              rb2 = p3.tile([128, SQB], F32, tag="rb2")
                              nc.gpsimd.partition_broadcast(rb2[:], rc[:])
                              sqsl = slice(sqb * SQB, (sqb + 1) * SQB)
                              ao32 = p3.tile([128, SQB], F32, tag="ao32")
                              nc.vector.tensor_mul(ao32[:], av[:], rb2[:])
                              nc.vector.tensor_copy(aoh8[:, sqsl], ao32[:])
                              nc.vector.tensor_tensor(
                                  aol8[:, sqsl], ao32[:], aoh8[:, sqsl],
                                  mybir.AluOpType.subtract)
                              if i == 0 and sqb == 0:
                                  # off the critical path: the b=1 norm
                                  # factors and the second pair's prep
                                  emit_rb(1)
                                  preps[1] = prep(1)
                                  # prefetch of the Wout prefix for phase 4
                                  for nb in range(WOSL // 512):
                                      nc.sync.dma_start(
                                          wot0[:, :, :,
                                               nb * 512:(nb + 1) * 512],
                                          w3o[:, :, :,
                                              nb * 512:(nb + 1) * 512])
                          nc.gpsimd.dma_start(
                              aosc8[hh * 256 + 128:hh * 256 + 256,
                                    b * S:(b + 1) * S], aoh8[:])
                          nc.gpsimd.dma_start(
                              aosc8[hh * 256:hh * 256 + 128,
                                    b * S:(b + 1) * S], aol8[:])

                # -------------- phase 4: partial output projection ----------
                NWO0 = WOSL // 512
                with (
                  tc.tile_pool(name="wo", bufs=1) as wo,
                  tc.tile_pool(name="p4", bufs=4) as p4,
                  tc.tile_pool(name="oe", bufs=2) as oep,
                  tc.tile_pool(name="ps4", bufs=4, space="PSUM") as ps4,
                ):
                  ao4 = aosc8.rearrange("(c j p) t -> p c j t", p=128,
                                        j=2)

                  def load_aot(tt):
                      aot = p4.tile([128, HPC, 2, 128], F8, tag="aot",
                                    name="aot")
                      nc.sync.dma_start(
                          aot[:], ao4[:, :, :, tt * 128:(tt + 1) * 128])
                      return aot

                  aot_next = load_aot(0)   # first attn-out tile before W bulk
                  wot = None
                  if DIM > WOSL:
                      wot = wo.tile([128, 6, 2, DIM - WOSL], F8, tag="wot")
                      for nb in range(NWO0, ONB):
                          nc.sync.dma_start(
                              wot[:, :, :, (nb - NWO0) * 512:
                                  (nb - NWO0 + 1) * 512],
                              w3o[:, :, :, nb * 512:(nb + 1) * 512])
                  for tt in range(NT // 128):
                      aot = aot_next
                      if tt + 1 < NT // 128:
                          aot_next = load_aot(tt + 1)
                      orow = oep.tile([128, DIM], F32, tag="orow")
                      for nb in range(ONB):
                          if nb < NWO0:
                              wsl = wot0[:, :, :, nb * 512:(nb + 1) * 512]
                          else:
                              wsl = wot[:, :, :, (nb - NWO0) * 512:
                                        (nb - NWO0 + 1) * 512]
                          po = ps4.tile([128, 512], F32, tag="po")
                          # 8 DoubleRow passes: 2 hi*hi chunk-pairs, 5
                          # crosses, 1 fused hi*hi+lo*lo for the odd chunk
                          nc.tensor.matmul(
                              po[:], aot[:, 0:2, 1, :], wsl[:, 0:2, 0, :],
                              start=True, stop=False, perf_mode=DRM)
                          nc.tensor.matmul(
                              po[:], aot[:, 2:4, 1, :], wsl[:, 2:4, 0, :],
                              start=False, stop=False, perf_mode=DRM)
                          for ch in range(HPC):
                              nc.tensor.matmul(
                                  po[:], aot[:, ch, :, :], wsl[:, ch, :, :],
                                  start=False, stop=False, perf_mode=DRM)
                          nc.tensor.matmul(
                              po[:], aot[:, 4, :, :], wsl[:, 5, :, :],
                              start=False, stop=True, perf_mode=DRM)
                          nc.vector.tensor_scalar_mul(
                              orow[:, nb * 512:(nb + 1) * 512], po[:],
                              OUT_DESC)
                      nc.sync.dma_start(
                          outp[tt * 128:(tt + 1) * 128, :], orow[:])
              _rb_ctx.close()
    nc.finalize()
    return nc


_PROGRAM_CACHE = {}


def _get_program(S, DIM, H):
    key = (S, DIM, H)
    if key not in _PROGRAM_CACHE:
        _PROGRAM_CACHE[key] = build_program(S, DIM, H)
    return _PROGRAM_CACHE[key]


def _split8(x, scale):
    xs = (np.asarray(x, np.float32) * np.float32(scale))
    hi = xs.astype(E4NP)
    lo = (xs - hi.astype(np.float32)).astype(E4NP)
    return hi, lo


def _pack_w(Wslice, scale):
    # [DIM, n] -> [(c j p), n] with j=(hi, lo)
    wh, wl = _split8(Wslice, scale)
    d, n = Wslice.shape
    w8 = np.empty((d // 128, 2, 128, n), E4NP)
    w8[:, 0] = wh.reshape(d // 128, 128, n)
    w8[:, 1] = wl.reshape(d // 128, 128, n)
    return np.ascontiguousarray(w8.reshape(d * 2, n))


def make_in_maps(S, DIM, H, hidden_cond, hidden_uncond, cos_freqs, sin_freqs,
                 Wqkv, bqkv, wq_norm, wk_norm, Wout, bout):
    HD = 128
    HPC = H // NCORES
    CW = HPC * HD
    NT = 2 * S
    h = np.concatenate([np.asarray(hidden_cond), np.asarray(hidden_uncond)],
                       axis=0).reshape(NT, DIM)
    hT = np.ascontiguousarray(h.T)
    hh, hl = _split8(hT, SH)
    hT8 = np.empty((DIM // 128, 2, 128, NT), E4NP)
    hT8[:, 0] = hl.reshape(DIM // 128, 128, NT)   # j=0: lo
    hT8[:, 1] = hh.reshape(DIM // 128, 128, NT)   # j=1: hi
    hT8 = np.ascontiguousarray(hT8.reshape(DIM * 2, NT))
    cosT = np.ascontiguousarray(
        np.asarray(cos_freqs).T.astype(ml_dtypes.bfloat16))
    sinT = np.asarray(sin_freqs).T  # [128, S]
    HF = HD // 2
    sinrT = np.concatenate([sinT[HF:], -sinT[:HF]], axis=0)
    sinrT = np.ascontiguousarray(sinrT.astype(ml_dtypes.bfloat16))
    Wqkv = np.asarray(Wqkv)
    bqkv = np.asarray(bqkv)
    wq_norm = np.asarray(wq_norm)
    wk_norm = np.asarray(wk_norm)
    Wout = np.asarray(Wout)

    in_maps = []
    for c in range(NCORES):
        sl = slice(c * CW, (c + 1) * CW)
        bq_c = (bqkv[0 * DIM:1 * DIM][sl] * (SH * SW)).astype(np.float32) \
            .reshape(HPC, HD).T
        bk_c = (bqkv[1 * DIM:2 * DIM][sl] * (SH * SW)).astype(np.float32) \
            .reshape(HPC, HD).T
        # out-proj rows (slot, j, p): slots 0..4 = chunk (hi,lo), slot 5 =
        # chunk 4 as (lo,hi) so its hi*hi+lo*lo fuses into one DoubleRow
        woh, wol = _split8(Wout[sl, :], SWO)
        wo8 = np.empty((6, 2, 128, DIM), E4NP)
        for s in range(HPC):
            wo8[s, 0] = woh[s * 128:(s + 1) * 128]
            wo8[s, 1] = wol[s * 128:(s + 1) * 128]
        wo8[5, 0] = wol[4 * 128:5 * 128]
        wo8[5, 1] = woh[4 * 128:5 * 128]
        in_maps.append({
            "hT8": hT8,
            "wq8": _pack_w(Wqkv[:, 0 * DIM:1 * DIM][:, sl], SW),
            "wk8": _pack_w(Wqkv[:, 1 * DIM:2 * DIM][:, sl], SW),
            "wv8": _pack_w(Wqkv[:, 2 * DIM:3 * DIM][:, sl], SW),
            "bq": np.ascontiguousarray(bq_c),
            "bk": np.ascontiguousarray(bk_c),
            "wqn": np.ascontiguousarray(wq_norm[sl].reshape(HPC, HD).T
                                        .astype(np.float32)),
            "wkn": np.ascontiguousarray(wk_norm[sl].reshape(HPC, HD).T
                                        .astype(np.float32)),
            "cosT": cosT,
            "sinrT": sinrT,
            "wo8": np.ascontiguousarray(wo8.reshape(6 * 2 * 128, DIM)),
        })
    return in_maps


def run(S, DIM, H, inputs):
    nc = _get_program(S, DIM, H)
    in_maps = make_in_maps(S, DIM, H, **inputs)
    res = run_bass_kernel_spmd(nc, in_maps, list(range(NCORES)))
    partial = np.zeros((2 * S, DIM), np.float64)
    for r in res.results:
        partial += r["outp"].astype(np.float64)
    # the v-bias contribution: softmax rows sum to 1, so attn(v + 1*bv) =
    # attn(v) + 1*bv, and bv flows through Wout as a constant per-channel term
    bv_full = np.asarray(inputs["bqkv"])[2 * DIM:3 * DIM].astype(np.float64)
    const_bias = bv_full @ np.asarray(inputs["Wout"]).astype(np.float64) \
        + np.asarray(inputs["bout"])
    out = (partial + const_bias[None, :]).astype(np.float32)
    out = out.reshape(2, 1, S, DIM)
    return out[0], out[1]


def kernel(hidden_cond, hidden_uncond, cos_freqs, sin_freqs,
           Wqkv, bqkv, wq_norm, wk_norm, Wout, bout):
    B, S, DIM = np.asarray(hidden_cond).shape
    assert B == 1
    H = DIM // 128
    return run(S, DIM, H, dict(
        hidden_cond=hidden_cond, hidden_uncond=hidden_uncond,
        cos_freqs=cos_freqs, sin_freqs=sin_freqs, Wqkv=Wqkv, bqkv=bqkv,
        wq_norm=wq_norm, wk_norm=wk_norm, Wout=Wout, bout=bout))

